# revision 10
# baseline (speedup 1.0000x reference)
"""GQA kernel for Trainium2, 8 NeuronCores.

Sharding: core c = b*4 + g  handles batch b, kv-head g (4 query heads).
Each core computes (bf16 matmuls, fp32 PSUM):
  Q_g^T = Wq_g @ x_q^T   [4 heads][128, S]  (1/sqrt(D) folded into Wq host-side)
  K_g^T = Wk_g @ x_k^T   [128, S]
  V_g   = via V^T then DMA-xbar transpose   [S, D]
  per q-chunk (512), per head, ascending k-tile PAIRS with causal diagonal
  subranges ([128,1024] score tiles -> one exp per pair):
    S^T = K_kt @ Q^T  (only q >= k columns on diagonal tiles)
    mask add only on the single 128x128 triangular diagonal block
    p = exp(S^T) kept in SBUF (p_all) -> PV accumulates o^T
  l row-sums = ones^T @ p as a head-end matmul burst (PSUM bank borrowed
  from the shared accumulator pool), r = recip_approx_fast(l), broadcast
  across partitions with a stride-0 DMA, o_norm^T = o^T * r.
  out_partial = o_norm @ Wo_g.T stored bf16; host sums 4 partials/batch.

Program interleaves x-chunk DMA loads, projections, attention heads and
out-projection s-tiles so the PE always has ready matmul work while the
scalar engine grinds exps (PE idle windows re-throttle the HAM clock gate
to half rate, which is the main perf cliff on trn2).
"""

import sys

import numpy as np

for _p in ("/opt/trn_rl_repo",):
    if _p not in sys.path:
        sys.path.insert(0, _p)

import ml_dtypes

import concourse.bass as bass
import concourse.mybir as mybir
from concourse import bacc
from concourse.bass_utils import run_bass_kernel_spmd
from concourse.tile import TileContext

B, S, E = 2, 2048, 2048
H, HKV = 16, 4
D = E // H  # 128
G = H // HKV  # 4 query heads per kv head
GD = G * D  # 512
NCORES = B * HKV  # 8
SC = 512  # q-chunk width
NSC = S // SC  # 4
NET = E // 128  # 16 e-tiles (contraction)
NKT = S // 128  # 16 k-tiles
SCALE = 1.0 / float(np.sqrt(D))

F32 = mybir.dt.float32
BF16 = mybir.dt.bfloat16
AF = mybir.ActivationFunctionType
NPBF = np.dtype(ml_dtypes.bfloat16)


def build_nc():
    nc = bacc.Bacc()
    xq = nc.declare_dram_parameter("xq", [E, S], BF16, isOutput=False)  # query[b].T
    xk = nc.declare_dram_parameter("xk", [E, S], BF16, isOutput=False)  # key[b].T
    xv = nc.declare_dram_parameter("xv", [E, S], BF16, isOutput=False)  # value[b].T
    wq = nc.declare_dram_parameter("wq", [E, GD], BF16, isOutput=False)
    wk = nc.declare_dram_parameter("wk", [E, D], BF16, isOutput=False)
    wv = nc.declare_dram_parameter("wv", [E, D], BF16, isOutput=False)
    wo = nc.declare_dram_parameter("wo", [GD, E], BF16, isOutput=False)
    msk = nc.declare_dram_parameter("msk", [128, 128], F32, isOutput=False)
    out = nc.declare_dram_parameter("out", [S, E], BF16, isOutput=True)

    def tiled_ap(dram, rowstride, ntile, ncol):
        # [128, ntile, ncol] view of a row-major DRAM [ntile*128, ncol] tensor
        base = dram[:, :]
        return bass.AP(
            tensor=base.tensor,
            offset=base.offset,
            ap=[[rowstride, 128], [128 * rowstride, ntile], [1, ncol]],
        )

    def x_chunk_ap(dram, tg, c):
        # e-tiles 4*tg..4*tg+3, s-columns [c*SC, (c+1)*SC)
        base = dram[:, :]
        return bass.AP(
            tensor=base.tensor,
            offset=base.offset + tg * 4 * 128 * S + c * SC,
            ap=[[S, 128], [128 * S, 4], [1, SC]],
        )

    with TileContext(nc) as tc:
        with (
            tc.tile_pool(name="singles", bufs=1) as singles,
            tc.tile_pool(name="xs", bufs=6) as xsp,
            tc.tile_pool(name="vtp", bufs=2) as vtp,
            tc.tile_pool(name="rpp", bufs=2) as rpp,
            tc.tile_pool(name="rbp", bufs=2) as rbp,
            tc.tile_pool(name="obp", bufs=2) as obp,
            tc.tile_pool(name="drp", bufs=2, space="DRAM") as drp,
            tc.tile_pool(name="pacc", bufs=2, space="PSUM") as pacc,
            tc.tile_pool(name="sacc", bufs=2, space="PSUM") as sacc,
            tc.tile_pool(name="ops", bufs=2, space="PSUM") as ops,
        ):
            # ---- resident weights / constants ----
            wq_sb = singles.tile([128, NET, GD], BF16)
            wk_sb = singles.tile([128, NET, D], BF16)
            wv_sb = singles.tile([128, NET, D], BF16)
            wo_sb = singles.tile([128, G, E], BF16)
            mask_sb = singles.tile([128, 128], F32)
            ones_bf = singles.tile([128, 128], BF16)
            ones_f = singles.tile([128, 128], F32)
            qT = singles.tile([128, G, S], BF16)
            kT = singles.tile([128, S], BF16)
            v_sb = singles.tile([128, NKT, D], BF16)
            onrm = singles.tile([128, G, 2, SC], BF16)  # 2-chunk rotation
            p_all = singles.tile([128, NKT * SC], BF16)

            nc.vector.memset(ones_f, 1.0)
            nc.scalar.activation(out=ones_bf[:], in_=ones_f[:], func=AF.Copy)
            nc.sync.dma_start(out=wk_sb[:], in_=tiled_ap(wk, D, NET, D))
            nc.sync.dma_start(out=wv_sb[:], in_=tiled_ap(wv, D, NET, D))
            nc.sync.dma_start(out=wq_sb[:], in_=tiled_ap(wq, GD, NET, GD))
            nc.sync.dma_start(out=wo_sb[:], in_=tiled_ap(wo, E, G, E))
            nc.sync.dma_start(out=mask_sb[:], in_=msk[:, :])

            def load(c):
                tiles = {"xk": [], "xv": [], "xq": []}
                for nm, dram in (("xk", xk), ("xv", xv), ("xq", xq)):
                    for tg in range(4):
                        t = xsp.tile(
                            [128, 4, SC], BF16, tag=nm, name=f"{nm}{c}{tg}"
                        )
                        nc.sync.dma_start(out=t[:], in_=x_chunk_ap(dram, tg, c))
                        tiles[nm].append(t)
                return tiles

            def proj_k(c, tiles):
                csl = slice(c * SC, (c + 1) * SC)
                xk_t = tiles["xk"]
                ps = pacc.tile([128, SC], F32, tag="acc", name="psk")
                for t in range(NET):
                    nc.tensor.matmul(
                        ps[:],
                        lhsT=wk_sb[:, t, :],
                        rhs=xk_t[t // 4][:, t % 4, :],
                        start=(t == 0),
                        stop=(t == NET - 1),
                    )
                nc.vector.tensor_copy(out=kT[:, csl], in_=ps[:])

            def proj_v(c, tiles):
                xv_t = tiles["xv"]
                ps = pacc.tile([128, SC], F32, tag="acc", name="psv")
                for t in range(NET):
                    nc.tensor.matmul(
                        ps[:],
                        lhsT=wv_sb[:, t, :],
                        rhs=xv_t[t // 4][:, t % 4, :],
                        start=(t == 0),
                        stop=(t == NET - 1),
                    )
                vt_tmp = vtp.tile([128, SC], BF16, tag="vt", name="vt")
                nc.scalar.activation(out=vt_tmp[:], in_=ps[:], func=AF.Copy)
                for i in range(SC // 128):
                    nc.sync.dma_start_transpose(
                        out=v_sb[:, c * 4 + i, :],
                        in_=vt_tmp[:, i * 128 : (i + 1) * 128],
                    )

            def proj_q(c, tiles, h):
                csl = slice(c * SC, (c + 1) * SC)
                xq_t = tiles["xq"]
                ps = pacc.tile([128, SC], F32, tag="acc", name="psq")
                for t in range(NET):
                    nc.tensor.matmul(
                        ps[:],
                        lhsT=wq_sb[:, t, h * D : (h + 1) * D],
                        rhs=xq_t[t // 4][:, t % 4, :],
                        start=(t == 0),
                        stop=(t == NET - 1),
                    )
                nc.scalar.activation(out=qT[:, h, csl], in_=ps[:], func=AF.Copy)

            def attn_head(qc, h):
                qbase = qc * SC
                nkt = 4 * qc + 4  # causal k-tiles, ascending, in pairs
                o_ps = ops.tile([128, SC], F32, tag="o", name="ops")
                for kp in range(nkt // 2):
                    kts = (2 * kp, 2 * kp + 1)
                    s2 = sacc.tile([128, 2 * SC], F32, tag="s", name="sps")
                    offs = []
                    for i, kt in enumerate(kts):
                        j = kt - 4 * qc
                        qoff = 128 * j if j >= 0 else 0
                        offs.append(qoff)
                        nc.tensor.matmul(
                            s2[:, i * SC + qoff : (i + 1) * SC],
                            lhsT=kT[:, kt * 128 : (kt + 1) * 128],
                            rhs=qT[:, h, qbase + qoff : qbase + SC],
                            start=True,
                            stop=True,
                        )
                        if j >= 0:
                            nc.vector.tensor_add(
                                s2[:, i * SC + qoff : i * SC + qoff + 128],
                                s2[:, i * SC + qoff : i * SC + qoff + 128],
                                mask_sb[:],
                            )
                    diag = kts[1] >= 4 * qc
                    if diag:
                        for i, kt in enumerate(kts):
                            nc.scalar.activation(
                                out=p_all[:, kt * SC + offs[i] : (kt + 1) * SC],
                                in_=s2[:, i * SC + offs[i] : (i + 1) * SC],
                                func=AF.Exp,
                            )
                    else:
                        nc.scalar.activation(
                            out=p_all[:, kts[0] * SC : (kts[1] + 1) * SC],
                            in_=s2[:],
                            func=AF.Exp,
                        )
                    for i, kt in enumerate(kts):
                        nc.tensor.matmul(
                            o_ps[:, offs[i] :],
                            lhsT=v_sb[:, kt, :],
                            rhs=p_all[:, kt * SC + offs[i] : (kt + 1) * SC],
                            start=(kt == 0),
                            stop=(kt == nkt - 1),
                            skip_group_check=True,
                        )
                # head-end l burst: l = ones^T @ p over all k-tiles
                l_acc = pacc.tile([128, SC], F32, tag="acc", name="lacc")
                for kt in range(nkt):
                    j = kt - 4 * qc
                    qoff = 128 * j if j >= 0 else 0
                    nc.tensor.matmul(
                        l_acc[0:1, qoff:],
                        lhsT=ones_bf[:, 0:1],
                        rhs=p_all[:, kt * SC + qoff : (kt + 1) * SC],
                        start=(kt == 0),
                        stop=(kt == nkt - 1),
                        skip_group_check=True,
                    )
                rp_f = rpp.tile([1, SC], F32, tag="rpf", name="rpf")
                nc.vector.reciprocal_approx_fast(out=rp_f[:], in_=l_acc[0:1, :])
                # broadcast r across partitions: DRAM bounce + stride-0 read,
                # dispatched from the idle gpsimd SWDGE queue to dodge
                # head-of-line blocking on the sync DMA queue.
                rp_d = drp.tile([1, SC], F32, tag="rpd", name="rpd")
                nc.gpsimd.dma_start(out=rp_d[:], in_=rp_f[:])
                rbc_s = rbp.tile([128, SC], F32, tag="rbs", name="rbs")
                rp_b = rp_d[:]
                nc.gpsimd.dma_start(
                    out=rbc_s[:],
                    in_=bass.AP(
                        tensor=rp_b.tensor,
                        offset=rp_b.offset,
                        ap=[[0, 128]] + list(rp_b.ap[1:]),
                    ),
                )
                nc.vector.tensor_mul(
                    onrm[:, h, qc % 2, :], o_ps[:], rbc_s[:]
                )

            def outproj_st(qc, sti):
                st = qc * 4 + sti
                stl = slice(st * 128, (st + 1) * 128)
                lsl = slice(sti * 128, (sti + 1) * 128)
                ob = obp.tile([128, E], BF16, tag="ob", name="ob")
                for ecp in range(2):
                    pse = [
                        pacc.tile([128, SC], F32, tag="acc", name=f"pso{e}")
                        for e in range(2)
                    ]
                    for h in range(G):
                        for e in range(2):
                            ec = 2 * ecp + e
                            nc.tensor.matmul(
                                pse[e][:],
                                lhsT=onrm[:, h, qc % 2, lsl],
                                rhs=wo_sb[:, h, ec * SC : (ec + 1) * SC],
                                start=(h == 0),
                                stop=(h == G - 1),
                            )
                    e0 = 2 * ecp * SC
                    nc.scalar.activation(
                        out=ob[:, e0 : e0 + SC], in_=pse[0][:], func=AF.Copy
                    )
                    nc.vector.tensor_copy(
                        out=ob[:, e0 + SC : e0 + 2 * SC], in_=pse[1][:]
                    )
                nc.sync.dma_start(out=out[stl, :], in_=ob[:])

            # schedule: per q-chunk cycle, alternate ACT-heavy attention
            # heads with PE-dense blocks (next chunk's projections, previous
            # chunk's out-projection s-tiles) so the PE never starves.
            tiles = load(0)
            proj_k(0, tiles)
            proj_v(0, tiles)
            for h in range(G):
                proj_q(0, tiles, h)
            tiles = load(1)
            for qc in range(NSC):
                pe_blocks = []
                if qc < NSC - 1:
                    nt = tiles
                    pe_blocks.append(lambda nt=nt: proj_k(qc + 1, nt))
                    pe_blocks.append(lambda nt=nt: proj_v(qc + 1, nt))
                    for h in range(G):
                        pe_blocks.append(
                            lambda nt=nt, h=h: proj_q(qc + 1, nt, h)
                        )
                if qc > 0:
                    for sti in range(4):
                        pe_blocks.append(
                            lambda sti=sti: outproj_st(qc - 1, sti)
                        )
                nblk = len(pe_blocks)
                bi = 0
                for h in range(G):
                    attn_head(qc, h)
                    take = (nblk * (h + 1)) // G - bi
                    for _ in range(take):
                        pe_blocks[bi]()
                        bi += 1
                if qc < NSC - 2:
                    tiles = load(qc + 2)
            for sti in range(4):
                outproj_st(NSC - 1, sti)
    nc.compile()
    return nc


_NC_CACHE = None


def _get_nc():
    global _NC_CACHE
    if _NC_CACHE is None:
        _NC_CACHE = build_nc()
    return _NC_CACHE


def _prep_in_maps(query, key, value, attn_mask, Wq, Wk, Wv, Wo):
    query = np.asarray(query, dtype=np.float32)
    key = np.asarray(key, dtype=np.float32)
    value = np.asarray(value, dtype=np.float32)
    Wq = np.asarray(Wq, dtype=np.float32)
    Wk = np.asarray(Wk, dtype=np.float32)
    Wv = np.asarray(Wv, dtype=np.float32)
    Wo = np.asarray(Wo, dtype=np.float32)
    am = np.asarray(attn_mask)

    xqT = [np.ascontiguousarray(query[b].T).astype(NPBF) for b in range(B)]
    xkT = [np.ascontiguousarray(key[b].T).astype(NPBF) for b in range(B)]
    xvT = [np.ascontiguousarray(value[b].T).astype(NPBF) for b in range(B)]

    # single 128x128 additive mask for the true diagonal block, [k, q] layout
    m0 = np.asarray(am[0, 0, :128, :128], dtype=np.float32)  # [q, k]
    msk_np = np.ascontiguousarray((m0.T - 1.0) * 1e9)

    in_maps = []
    for b in range(B):
        for g in range(HKV):
            in_maps.append(
                {
                    "xq": xqT[b],
                    "xk": xkT[b],
                    "xv": xvT[b],
                    "wq": np.ascontiguousarray(
                        Wq[g * GD : (g + 1) * GD, :].T * SCALE
                    ).astype(NPBF),
                    "wk": np.ascontiguousarray(
                        Wk[g * D : (g + 1) * D, :].T
                    ).astype(NPBF),
                    "wv": np.ascontiguousarray(
                        Wv[g * D : (g + 1) * D, :].T
                    ).astype(NPBF),
                    "wo": np.ascontiguousarray(
                        Wo[:, g * GD : (g + 1) * GD].T
                    ).astype(NPBF),
                    "msk": msk_np,
                }
            )
    return in_maps


def _run(inputs, trace=False, **kw):
    nc = _get_nc()
    in_maps = _prep_in_maps(**inputs)
    res = run_bass_kernel_spmd(
        nc, in_maps, list(range(NCORES)), trace=trace, **kw
    )
    outs = [np.asarray(r["out"]) for r in res.results]
    full = np.empty((B, S, E), dtype=np.float32)
    for b in range(B):
        acc = outs[b * HKV].astype(np.float32)
        for g in range(1, HKV):
            acc = acc + outs[b * HKV + g].astype(np.float32)
        full[b] = acc
    return full, res


def kernel(**inputs):
    full, _ = _run(inputs, trace=False)
    return full


# revision 11
# speedup vs baseline: 1.0556x; 1.0556x over previous
"""GQA kernel for Trainium2, 8 NeuronCores.

Sharding: core c = b*4 + g  handles batch b, kv-head g (4 query heads).
Each core computes (bf16 matmuls, fp32 PSUM):
  Q_g^T = Wq_g @ x_q^T   [4 heads][128, S]  (1/sqrt(D) folded into Wq host-side)
  K_g^T = Wk_g @ x_k^T   [128, S]
  V_g   = via V^T then DMA-xbar transpose   [S, D]
  per q-chunk (512), per head, ascending k-tile PAIRS with causal diagonal
  subranges ([128,1024] score tiles -> one exp per pair):
    S^T = K_kt @ Q^T  (only q >= k columns on diagonal tiles)
    mask add only on the single 128x128 triangular diagonal block
    p = exp(S^T) kept in SBUF (p_all) -> PV accumulates o^T
  l row-sums = ones^T @ p as a head-end matmul burst (PSUM bank borrowed
  from the shared accumulator pool), r = recip_approx_fast(l), broadcast
  across partitions with a stride-0 DMA, o_norm^T = o^T * r.
  out_partial = o_norm @ Wo_g.T stored bf16; host sums 4 partials/batch.

Program interleaves x-chunk DMA loads, projections, attention heads and
out-projection s-tiles so the PE always has ready matmul work while the
scalar engine grinds exps (PE idle windows re-throttle the HAM clock gate
to half rate, which is the main perf cliff on trn2).
"""

import sys

import numpy as np

for _p in ("/opt/trn_rl_repo",):
    if _p not in sys.path:
        sys.path.insert(0, _p)

import ml_dtypes

import concourse.bass as bass
import concourse.mybir as mybir
from concourse import bacc
from concourse.bass_utils import run_bass_kernel_spmd
from concourse.tile import TileContext

B, S, E = 2, 2048, 2048
H, HKV = 16, 4
D = E // H  # 128
G = H // HKV  # 4 query heads per kv head
GD = G * D  # 512
NCORES = B * HKV  # 8
SC = 512  # q-chunk width
NSC = S // SC  # 4
NET = E // 128  # 16 e-tiles (contraction)
NKT = S // 128  # 16 k-tiles
SCALE = 1.0 / float(np.sqrt(D))

F32 = mybir.dt.float32
BF16 = mybir.dt.bfloat16
AF = mybir.ActivationFunctionType
NPBF = np.dtype(ml_dtypes.bfloat16)


def build_nc():
    nc = bacc.Bacc()
    xq = nc.declare_dram_parameter("xq", [E, S], BF16, isOutput=False)  # query[b].T
    xk = nc.declare_dram_parameter("xk", [E, S], BF16, isOutput=False)  # key[b].T
    xv = nc.declare_dram_parameter("xv", [E, S], BF16, isOutput=False)  # value[b].T
    wq = nc.declare_dram_parameter("wq", [E, GD], BF16, isOutput=False)
    wk = nc.declare_dram_parameter("wk", [E, D], BF16, isOutput=False)
    wv = nc.declare_dram_parameter("wv", [E, D], BF16, isOutput=False)
    wo = nc.declare_dram_parameter("wo", [GD, E], BF16, isOutput=False)
    msk = nc.declare_dram_parameter("msk", [128, 128], F32, isOutput=False)
    out = nc.declare_dram_parameter("out", [S, E], BF16, isOutput=True)

    def tiled_ap(dram, rowstride, ntile, ncol):
        # [128, ntile, ncol] view of a row-major DRAM [ntile*128, ncol] tensor
        base = dram[:, :]
        return bass.AP(
            tensor=base.tensor,
            offset=base.offset,
            ap=[[rowstride, 128], [128 * rowstride, ntile], [1, ncol]],
        )

    def x_chunk_ap(dram, tg, c):
        # e-tiles 4*tg..4*tg+3, s-columns [c*SC, (c+1)*SC)
        base = dram[:, :]
        return bass.AP(
            tensor=base.tensor,
            offset=base.offset + tg * 4 * 128 * S + c * SC,
            ap=[[S, 128], [128 * S, 4], [1, SC]],
        )

    with TileContext(nc) as tc:
        with (
            tc.tile_pool(name="singles", bufs=1) as singles,
            tc.tile_pool(name="xs", bufs=6) as xsp,
            tc.tile_pool(name="vtp", bufs=2) as vtp,
            tc.tile_pool(name="rpp", bufs=2) as rpp,
            tc.tile_pool(name="rbp", bufs=2) as rbp,
            tc.tile_pool(name="obp", bufs=2) as obp,
            tc.tile_pool(name="drp", bufs=2, space="DRAM") as drp,
            tc.tile_pool(name="pacc", bufs=2, space="PSUM") as pacc,
            tc.tile_pool(name="sacc", bufs=2, space="PSUM") as sacc,
            tc.tile_pool(name="ops", bufs=2, space="PSUM") as ops,
        ):
            # ---- resident weights / constants ----
            wq_sb = singles.tile([128, NET, GD], BF16)
            wk_sb = singles.tile([128, NET, D], BF16)
            wv_sb = singles.tile([128, NET, D], BF16)
            wo_sb = singles.tile([128, G, E], BF16)
            mask_sb = singles.tile([128, 128], F32)
            ones_bf = singles.tile([128, 128], BF16)
            ones_f = singles.tile([128, 128], F32)
            qT = singles.tile([128, G, S], BF16)
            kT = singles.tile([128, S], BF16)
            v_sb = singles.tile([128, NKT, D], BF16)
            onrm = singles.tile([128, G, 2, SC], BF16)  # 2-chunk rotation
            p_all = singles.tile([128, NKT * SC], BF16)

            nc.vector.memset(ones_f, 1.0)
            nc.scalar.activation(out=ones_bf[:], in_=ones_f[:], func=AF.Copy)
            nc.sync.dma_start(out=wk_sb[:], in_=tiled_ap(wk, D, NET, D))
            nc.sync.dma_start(out=wv_sb[:], in_=tiled_ap(wv, D, NET, D))
            nc.sync.dma_start(out=wq_sb[:], in_=tiled_ap(wq, GD, NET, GD))
            nc.sync.dma_start(out=wo_sb[:], in_=tiled_ap(wo, E, G, E))
            nc.sync.dma_start(out=mask_sb[:], in_=msk[:, :])

            def load(c):
                tiles = {"xk": [], "xv": [], "xq": []}
                for nm, dram in (("xk", xk), ("xv", xv), ("xq", xq)):
                    for tg in range(4):
                        t = xsp.tile(
                            [128, 4, SC], BF16, tag=nm, name=f"{nm}{c}{tg}"
                        )
                        nc.sync.dma_start(out=t[:], in_=x_chunk_ap(dram, tg, c))
                        tiles[nm].append(t)
                return tiles

            def proj_k(c, tiles):
                csl = slice(c * SC, (c + 1) * SC)
                xk_t = tiles["xk"]
                ps = pacc.tile([128, SC], F32, tag="acc", name="psk")
                for t in range(NET):
                    nc.tensor.matmul(
                        ps[:],
                        lhsT=wk_sb[:, t, :],
                        rhs=xk_t[t // 4][:, t % 4, :],
                        start=(t == 0),
                        stop=(t == NET - 1),
                    )
                nc.vector.tensor_copy(out=kT[:, csl], in_=ps[:])

            def proj_v(c, tiles):
                xv_t = tiles["xv"]
                ps = pacc.tile([128, SC], F32, tag="acc", name="psv")
                for t in range(NET):
                    nc.tensor.matmul(
                        ps[:],
                        lhsT=wv_sb[:, t, :],
                        rhs=xv_t[t // 4][:, t % 4, :],
                        start=(t == 0),
                        stop=(t == NET - 1),
                    )
                vt_tmp = vtp.tile([128, SC], BF16, tag="vt", name="vt")
                nc.scalar.activation(out=vt_tmp[:], in_=ps[:], func=AF.Copy)
                for i in range(SC // 128):
                    nc.sync.dma_start_transpose(
                        out=v_sb[:, c * 4 + i, :],
                        in_=vt_tmp[:, i * 128 : (i + 1) * 128],
                    )

            def proj_q(c, tiles, h):
                csl = slice(c * SC, (c + 1) * SC)
                xq_t = tiles["xq"]
                ps = pacc.tile([128, SC], F32, tag="acc", name="psq")
                for t in range(NET):
                    nc.tensor.matmul(
                        ps[:],
                        lhsT=wq_sb[:, t, h * D : (h + 1) * D],
                        rhs=xq_t[t // 4][:, t % 4, :],
                        start=(t == 0),
                        stop=(t == NET - 1),
                    )
                nc.scalar.activation(out=qT[:, h, csl], in_=ps[:], func=AF.Copy)

            def attn_head(qc, h):
                qbase = qc * SC
                nkt = 4 * qc + 4  # causal k-tiles, ascending, in pairs
                o_ps = ops.tile([128, SC], F32, tag="o", name="ops")
                for kp in range(nkt // 2):
                    kts = (2 * kp, 2 * kp + 1)
                    s2 = sacc.tile([128, 2 * SC], F32, tag="s", name="sps")
                    offs = []
                    for i, kt in enumerate(kts):
                        j = kt - 4 * qc
                        qoff = 128 * j if j >= 0 else 0
                        offs.append(qoff)
                        nc.tensor.matmul(
                            s2[:, i * SC + qoff : (i + 1) * SC],
                            lhsT=kT[:, kt * 128 : (kt + 1) * 128],
                            rhs=qT[:, h, qbase + qoff : qbase + SC],
                            start=True,
                            stop=True,
                        )
                        if j >= 0:
                            nc.vector.tensor_add(
                                s2[:, i * SC + qoff : i * SC + qoff + 128],
                                s2[:, i * SC + qoff : i * SC + qoff + 128],
                                mask_sb[:],
                            )
                    diag = kts[1] >= 4 * qc
                    if diag:
                        for i, kt in enumerate(kts):
                            nc.scalar.activation(
                                out=p_all[:, kt * SC + offs[i] : (kt + 1) * SC],
                                in_=s2[:, i * SC + offs[i] : (i + 1) * SC],
                                func=AF.Exp,
                            )
                    else:
                        nc.scalar.activation(
                            out=p_all[:, kts[0] * SC : (kts[1] + 1) * SC],
                            in_=s2[:],
                            func=AF.Exp,
                        )
                    for i, kt in enumerate(kts):
                        nc.tensor.matmul(
                            o_ps[:, offs[i] :],
                            lhsT=v_sb[:, kt, :],
                            rhs=p_all[:, kt * SC + offs[i] : (kt + 1) * SC],
                            start=(kt == 0),
                            stop=(kt == nkt - 1),
                            skip_group_check=True,
                        )
                # head-end l burst: l = ones^T @ p over all k-tiles
                l_acc = pacc.tile([128, SC], F32, tag="acc", name="lacc")
                for kt in range(nkt):
                    j = kt - 4 * qc
                    qoff = 128 * j if j >= 0 else 0
                    nc.tensor.matmul(
                        l_acc[0:1, qoff:],
                        lhsT=ones_bf[:, 0:1],
                        rhs=p_all[:, kt * SC + qoff : (kt + 1) * SC],
                        start=(kt == 0),
                        stop=(kt == nkt - 1),
                        skip_group_check=True,
                    )
                rp_f = rpp.tile([1, SC], F32, tag="rpf", name="rpf")
                nc.vector.reciprocal_approx_fast(out=rp_f[:], in_=l_acc[0:1, :])
                # broadcast r across partitions: DRAM bounce + stride-0 read,
                # dispatched from the idle gpsimd SWDGE queue to dodge
                # head-of-line blocking on the sync DMA queue.
                rp_d = drp.tile([1, SC], F32, tag="rpd", name="rpd")
                nc.sync.dma_start(out=rp_d[:], in_=rp_f[:])
                rbc_s = rbp.tile([128, SC], F32, tag="rbs", name="rbs")
                rp_b = rp_d[:]
                nc.sync.dma_start(
                    out=rbc_s[:],
                    in_=bass.AP(
                        tensor=rp_b.tensor,
                        offset=rp_b.offset,
                        ap=[[0, 128]] + list(rp_b.ap[1:]),
                    ),
                )
                nc.vector.tensor_mul(
                    onrm[:, h, qc % 2, :], o_ps[:], rbc_s[:]
                )

            def outproj_st(qc, sti):
                st = qc * 4 + sti
                stl = slice(st * 128, (st + 1) * 128)
                lsl = slice(sti * 128, (sti + 1) * 128)
                ob = obp.tile([128, E], BF16, tag="ob", name="ob")
                for ecp in range(2):
                    pse = [
                        pacc.tile([128, SC], F32, tag="acc", name=f"pso{e}")
                        for e in range(2)
                    ]
                    for h in range(G):
                        for e in range(2):
                            ec = 2 * ecp + e
                            nc.tensor.matmul(
                                pse[e][:],
                                lhsT=onrm[:, h, qc % 2, lsl],
                                rhs=wo_sb[:, h, ec * SC : (ec + 1) * SC],
                                start=(h == 0),
                                stop=(h == G - 1),
                            )
                    e0 = 2 * ecp * SC
                    nc.scalar.activation(
                        out=ob[:, e0 : e0 + SC], in_=pse[0][:], func=AF.Copy
                    )
                    nc.vector.tensor_copy(
                        out=ob[:, e0 + SC : e0 + 2 * SC], in_=pse[1][:]
                    )
                nc.gpsimd.dma_start(out=out[stl, :], in_=ob[:])

            # schedule: per q-chunk cycle, alternate ACT-heavy attention
            # heads with PE-dense blocks (next chunk's projections, previous
            # chunk's out-projection s-tiles) so the PE never starves.
            tiles = load(0)
            proj_k(0, tiles)
            proj_v(0, tiles)
            for h in range(G):
                proj_q(0, tiles, h)
            tiles = load(1)
            for qc in range(NSC):
                pe_blocks = []
                if qc < NSC - 1:
                    nt = tiles
                    pe_blocks.append(lambda nt=nt: proj_k(qc + 1, nt))
                    pe_blocks.append(lambda nt=nt: proj_v(qc + 1, nt))
                    for h in range(G):
                        pe_blocks.append(
                            lambda nt=nt, h=h: proj_q(qc + 1, nt, h)
                        )
                if qc > 0:
                    for sti in range(4):
                        pe_blocks.append(
                            lambda sti=sti: outproj_st(qc - 1, sti)
                        )
                nblk = len(pe_blocks)
                bi = 0
                for h in range(G):
                    attn_head(qc, h)
                    take = (nblk * (h + 1)) // G - bi
                    for _ in range(take):
                        pe_blocks[bi]()
                        bi += 1
                if qc < NSC - 2:
                    tiles = load(qc + 2)
            for sti in range(4):
                outproj_st(NSC - 1, sti)
    nc.compile()
    return nc


_NC_CACHE = None


def _get_nc():
    global _NC_CACHE
    if _NC_CACHE is None:
        _NC_CACHE = build_nc()
    return _NC_CACHE


def _prep_in_maps(query, key, value, attn_mask, Wq, Wk, Wv, Wo):
    query = np.asarray(query, dtype=np.float32)
    key = np.asarray(key, dtype=np.float32)
    value = np.asarray(value, dtype=np.float32)
    Wq = np.asarray(Wq, dtype=np.float32)
    Wk = np.asarray(Wk, dtype=np.float32)
    Wv = np.asarray(Wv, dtype=np.float32)
    Wo = np.asarray(Wo, dtype=np.float32)
    am = np.asarray(attn_mask)

    xqT = [np.ascontiguousarray(query[b].T).astype(NPBF) for b in range(B)]
    xkT = [np.ascontiguousarray(key[b].T).astype(NPBF) for b in range(B)]
    xvT = [np.ascontiguousarray(value[b].T).astype(NPBF) for b in range(B)]

    # single 128x128 additive mask for the true diagonal block, [k, q] layout
    m0 = np.asarray(am[0, 0, :128, :128], dtype=np.float32)  # [q, k]
    msk_np = np.ascontiguousarray((m0.T - 1.0) * 1e9)

    in_maps = []
    for b in range(B):
        for g in range(HKV):
            in_maps.append(
                {
                    "xq": xqT[b],
                    "xk": xkT[b],
                    "xv": xvT[b],
                    "wq": np.ascontiguousarray(
                        Wq[g * GD : (g + 1) * GD, :].T * SCALE
                    ).astype(NPBF),
                    "wk": np.ascontiguousarray(
                        Wk[g * D : (g + 1) * D, :].T
                    ).astype(NPBF),
                    "wv": np.ascontiguousarray(
                        Wv[g * D : (g + 1) * D, :].T
                    ).astype(NPBF),
                    "wo": np.ascontiguousarray(
                        Wo[:, g * GD : (g + 1) * GD].T
                    ).astype(NPBF),
                    "msk": msk_np,
                }
            )
    return in_maps


def _run(inputs, trace=False, **kw):
    nc = _get_nc()
    in_maps = _prep_in_maps(**inputs)
    res = run_bass_kernel_spmd(
        nc, in_maps, list(range(NCORES)), trace=trace, **kw
    )
    outs = [np.asarray(r["out"]) for r in res.results]
    full = np.empty((B, S, E), dtype=np.float32)
    for b in range(B):
        acc = outs[b * HKV].astype(np.float32)
        for g in range(1, HKV):
            acc = acc + outs[b * HKV + g].astype(np.float32)
        full[b] = acc
    return full, res


def kernel(**inputs):
    full, _ = _run(inputs, trace=False)
    return full
